# revision 3
# baseline (speedup 1.0000x reference)
"""Trainium2 Bass kernel for nn_LinearAttention (B=8, C=256, H=W=64, 4 heads x 128).

Strategy
--------
Data-parallel over batch: each of the 8 NeuronCores processes one batch
element end-to-end (no collectives).

Per-core math (x is [C=256, n=4096], weights from the 1x1 convs):
    k^T = x^T @ w_k^T          [n, 512]   (n on partitions -> softmax-free layout)
    e   = exp(k^T)             (softmax without max-subtraction; |k| <~ 5)
    v^T = x^T @ w_v^T          [n, 512]
    ctx_h = e_h^T @ [v_h | 1]  [128, 129] accumulated over n-tiles on PSUM;
                               col 128 gives the softmax row-sums for free.
    ctx_h /= rowsum            (tiny [128,128] per-partition scale)
    M_h   = ctx_h^T @ w_q_h    [128, 256]
    W^T   = sum_h M_h @ w_out_h^T   [256, 256]  ("algebraic collapse":
            out = w_out @ (ctx^T @ (w_q @ x)) == (w_out ctx^T w_q) @ x)
    out   = W @ x + b          [256, 4096]

This removes the q / attention-out / final-projection streaming matmuls
(~1.1 GMAC/core) and replaces them with a single [256,256] @ [256,4096]
matmul. Matmuls run as float32r (fp32 operands at ~bf16 speed for free
dim >= 256); the context contraction uses bf16 operands with fp32 PSUM
accumulation.
"""

import numpy as np

HEADS = 4
DH = 128
C = 256
HID = 512
N = 4096
NT = N // 128  # 32 n-tiles
NCORES = 8

_BUILD_CACHE = {}


def _build_program():
    """Build + compile the SPMD Bass program (same NEFF for all 8 cores)."""
    from contextlib import ExitStack

    import concourse.bass as bass
    import concourse.tile as tile
    from concourse import bacc, mybir

    f32 = mybir.dt.float32
    f32r = mybir.dt.float32r
    bf16 = mybir.dt.bfloat16
    AFT = mybir.ActivationFunctionType

    nc = bacc.Bacc(
        "TRN2", target_bir_lowering=False, debug=False, num_devices=NCORES
    )

    x_d = nc.dram_tensor("x", [C, N], f32r, kind="ExternalInput").ap()
    wk_d = nc.dram_tensor("wk", [128, 2 * HID], f32r, kind="ExternalInput").ap()
    wv_d = nc.dram_tensor("wv", [128, 2 * HID], f32r, kind="ExternalInput").ap()
    wq_d = nc.dram_tensor("wq", [128, HEADS * C], f32r, kind="ExternalInput").ap()
    wo_d = nc.dram_tensor("wo", [128, HEADS * C], f32r, kind="ExternalInput").ap()
    bb_d = nc.dram_tensor("bb", [128, 2], f32, kind="ExternalInput").ap()
    out_d = nc.dram_tensor("out", [C, N], f32, kind="ExternalOutput").ap()

    with tile.TileContext(nc) as tc, ExitStack() as stack:
        const = stack.enter_context(tc.tile_pool(name="const", bufs=1))

        x_sb = const.tile([128, 2 * N], f32r)
        nc.sync.dma_start(x_sb[:, 0:N], x_d[0:128, :])
        nc.sync.dma_start(x_sb[:, N : 2 * N], x_d[128:C, :])
        wk_sb = const.tile([128, 2 * HID], f32r)
        nc.sync.dma_start(wk_sb[:], wk_d[:])
        wv_sb = const.tile([128, 2 * HID], f32r)
        nc.sync.dma_start(wv_sb[:], wv_d[:])
        wq_sb = const.tile([128, HEADS * C], f32r)
        nc.sync.dma_start(wq_sb[:], wq_d[:])
        wo_sb = const.tile([128, HEADS * C], f32r)
        nc.sync.dma_start(wo_sb[:], wo_d[:])
        bb_sb = const.tile([128, 2], f32)
        nc.sync.dma_start(bb_sb[:], bb_d[:])

        def xs(k, i):  # lhsT: x rows k-block, spatial tile i -> [128, 128]
            return x_sb[:, k * N + i * 128 : k * N + (i + 1) * 128]

        def xchunk(k, c):  # rhs: x rows k-block, 512-col chunk c
            return x_sb[:, k * N + c * 512 : k * N + (c + 1) * 512]

        rsum = const.tile([128, HEADS], f32)
        ctx_sb = const.tile([128, HEADS * 128], f32r)

        # ---- Phase 1: k^T/v^T projections + exp + context accumulation ----
        # ctx accumulators: one PSUM bank per head (start=True zeroes a whole
        # bank, so heads cannot share one).
        with tc.tile_pool(name="ctxp", bufs=1, space="PSUM") as ctxp, \
             tc.tile_pool(name="pkp", bufs=2, space="PSUM") as pkp, \
             tc.tile_pool(name="pvp", bufs=2, space="PSUM") as pvp, \
             tc.tile_pool(name="ekp", bufs=3) as ekp, \
             tc.tile_pool(name="vtp", bufs=3) as vtp:
            ctx_ps = [
                ctxp.tile([128, 129], f32, name=f"ctx{h}") for h in range(HEADS)
            ]

            def emit_ctx(ek, vt, i):
                for h in range(HEADS):
                    nc.tensor.matmul(
                        ctx_ps[h][:],
                        ek[:, h * 128 : (h + 1) * 128],
                        vt[:, h * 130 : h * 130 + 129],
                        start=(i == 0),
                        stop=(i == NT - 1),
                    )

            pending = None
            for i in range(NT):
                pk = pkp.tile([128, HID], f32, name="pk")
                pv = pvp.tile([128, HID], f32, name="pv")
                for k in range(2):
                    first, last = (k == 0), (k == 1)
                    nc.tensor.matmul(
                        pk[:],
                        xs(k, i),
                        wk_sb[:, k * HID : (k + 1) * HID],
                        start=first,
                        stop=last,
                    )
                    nc.tensor.matmul(
                        pv[:],
                        xs(k, i),
                        wv_sb[:, k * HID : (k + 1) * HID],
                        start=first,
                        stop=last,
                    )
                ek = ekp.tile([128, HID], bf16, name="ek")
                nc.scalar.activation(ek[:], pk[:], AFT.Exp)
                vt = vtp.tile([128, 4 * 130], bf16, name="vt")
                nc.vector.tensor_copy(
                    vt.rearrange("p (h c) -> p h c", c=130)[:, :, 0:128],
                    pv.rearrange("p (h c) -> p h c", c=128),
                )
                nc.gpsimd.memset(
                    vt.rearrange("p (h c) -> p h c", c=130)[:, :, 128:129], 1.0
                )
                # software-pipeline the context matmuls one tile behind so the
                # tensor engine never stalls on the exp/copy of the same tile
                if pending is not None:
                    emit_ctx(*pending)
                pending = (ek, vt, i)
            emit_ctx(*pending)

            # ---- normalize ctx while the accumulator banks are still open ----
            for h in range(HEADS):
                nc.vector.reciprocal(rsum[:, h : h + 1], ctx_ps[h][:, 128:129])
                nc.vector.tensor_scalar_mul(
                    ctx_sb[:, h * 128 : (h + 1) * 128],
                    ctx_ps[h][:, 0:128],
                    rsum[:, h : h + 1],
                )

        # ---- Phase 2: collapse weights, final matmul ----

        with tc.tile_pool(name="p2p", bufs=2, space="PSUM") as p2p, \
             tc.tile_pool(name="fop", bufs=3) as fop:
            # M_h = ctx_h^T @ w_q_h  -> [128, 256]
            m_sb = const.tile([128, HEADS * C], f32r)
            for h in range(HEADS):
                mp = p2p.tile([128, C], f32, name="mp")
                nc.tensor.matmul(
                    mp[:],
                    ctx_sb[:, h * 128 : (h + 1) * 128],
                    wq_sb[:, h * C : (h + 1) * C],
                )
                nc.vector.tensor_copy(m_sb[:, h * C : (h + 1) * C], mp[:])
            # W^T[ci-block m] = sum_h M_h[:, m-block]^T-contract w_out^T_h
            w_sb = const.tile([128, 2 * C], f32r)
            for m in range(2):
                wp = p2p.tile([128, C], f32, name="wp")
                for h in range(HEADS):
                    nc.tensor.matmul(
                        wp[:],
                        m_sb[:, h * C + m * 128 : h * C + m * 128 + 128],
                        wo_sb[:, h * C : (h + 1) * C],
                        start=(h == 0),
                        stop=(h == HEADS - 1),
                    )
                nc.vector.tensor_copy(w_sb[:, m * C : (m + 1) * C], wp[:])
            # out = W @ x + b, streamed over 8 chunks of 512 columns
            for c in range(8):
                for mo in range(2):
                    fp_ = p2p.tile([128, 512], f32, name="fp")
                    for k in range(2):
                        nc.tensor.matmul(
                            fp_[:],
                            w_sb[:, k * C + mo * 128 : k * C + mo * 128 + 128],
                            xchunk(k, c),
                            start=(k == 0),
                            stop=(k == 1),
                        )
                    fo = fop.tile([128, 512], f32, name=f"fo{mo}")
                    if mo == 0:
                        nc.scalar.activation(
                            fo[:], fp_[:], AFT.Identity, bias=bb_sb[:, 0:1]
                        )
                    else:
                        nc.vector.tensor_scalar_add(fo[:], fp_[:], bb_sb[:, 1:2])
                    nc.sync.dma_start(
                        out_d[mo * 128 : (mo + 1) * 128, c * 512 : (c + 1) * 512],
                        fo[:],
                    )

    nc.compile()
    return nc


def _get_program():
    if "nc" not in _BUILD_CACHE:
        _BUILD_CACHE["nc"] = _build_program()
    return _BUILD_CACHE["nc"]


def _pack_weights(w_qkv, w_out, b_out):
    w_q = np.ascontiguousarray(w_qkv[0:HID]).astype(np.float32)  # [512, 256]
    w_k = w_qkv[HID : 2 * HID]
    w_v = w_qkv[2 * HID : 3 * HID]

    def pack_T(w):  # w [512, 256] -> w.T [256, 512] -> [128, 2*512]
        return np.ascontiguousarray(
            w.T.reshape(2, 128, HID).transpose(1, 0, 2).reshape(128, 2 * HID)
        ).astype(np.float32)

    def pack_rows(w):  # w [512, 256] -> [128, 4*256], block h = rows h*128:+128
        return np.ascontiguousarray(
            w.reshape(HEADS, 128, C).transpose(1, 0, 2).reshape(128, HEADS * C)
        ).astype(np.float32)

    return {
        "wk": pack_T(w_k),
        "wv": pack_T(w_v),
        "wq": pack_rows(w_q),
        "wo": pack_rows(np.ascontiguousarray(w_out.T)),  # w_out.T [512, 256]
        "bb": np.ascontiguousarray(b_out.reshape(2, 128).T).astype(np.float32),
    }


def kernel(x, w_qkv, w_out, b_out):
    from concourse.bass_utils import run_bass_kernel_spmd

    x = np.asarray(x, dtype=np.float32)
    B = x.shape[0]
    assert B == NCORES and x.shape[1:] == (C, 64, 64)

    nc = _get_program()
    packed = _pack_weights(
        np.asarray(w_qkv, np.float32),
        np.asarray(w_out, np.float32),
        np.asarray(b_out, np.float32),
    )
    in_maps = [
        {"x": np.ascontiguousarray(x[b].reshape(C, N)), **packed} for b in range(B)
    ]
    res = run_bass_kernel_spmd(nc, in_maps, core_ids=list(range(NCORES)))
    out = np.stack([res.results[b]["out"] for b in range(B)], axis=0)
    return out.reshape(B, C, 64, 64).astype(np.float32)


# revision 5
# speedup vs baseline: 1.0875x; 1.0875x over previous
"""Trainium2 Bass kernel for nn_LinearAttention (B=8, C=256, H=W=64, 4 heads x 128).

Strategy
--------
Data-parallel over batch: each of the 8 NeuronCores processes one batch
element end-to-end (no collectives).

Per-core math (x is [C=256, n=4096], weights from the 1x1 convs):
    k^T = x^T @ w_k^T          [n, 512]   (n on partitions -> softmax-free layout)
    e   = exp(k^T)             (softmax without max-subtraction; |k| <~ 5)
    v^T = x^T @ w_v^T          [n, 512]
    ctx_h = e_h^T @ [v_h | 1]  [128, 129] accumulated over n-tiles on PSUM;
                               col 128 gives the softmax row-sums for free.
    ctx_h /= rowsum            (tiny [128,128] per-partition scale)
    M_h   = ctx_h^T @ w_q_h    [128, 256]
    W^T   = sum_h M_h @ w_out_h^T   [256, 256]  ("algebraic collapse":
            out = w_out @ (ctx^T @ (w_q @ x)) == (w_out ctx^T w_q) @ x)
    out   = W @ x + b          [256, 4096]

This removes the q / attention-out / final-projection streaming matmuls
(~1.1 GMAC/core) and replaces them with a single [256,256] @ [256,4096]
matmul. Matmuls run as float32r (fp32 operands at ~bf16 speed for free
dim >= 256); the context contraction uses bf16 operands with fp32 PSUM
accumulation.
"""

import numpy as np

HEADS = 4
DH = 128
C = 256
HID = 512
N = 4096
NT = N // 128  # 32 n-tiles
NCORES = 8

_BUILD_CACHE = {}


def _build_program():
    """Build + compile the SPMD Bass program (same NEFF for all 8 cores)."""
    from contextlib import ExitStack

    import concourse.bass as bass
    import concourse.tile as tile
    from concourse import bacc, mybir

    f32 = mybir.dt.float32
    f32r = mybir.dt.float32r
    bf16 = mybir.dt.bfloat16
    AFT = mybir.ActivationFunctionType

    nc = bacc.Bacc(
        "TRN2", target_bir_lowering=False, debug=False, num_devices=NCORES
    )

    x_d = nc.dram_tensor("x", [C, N], f32r, kind="ExternalInput").ap()
    wk_d = nc.dram_tensor("wk", [128, 2 * HID], f32r, kind="ExternalInput").ap()
    wv_d = nc.dram_tensor("wv", [128, 2 * HID], f32r, kind="ExternalInput").ap()
    wq_d = nc.dram_tensor("wq", [128, HEADS * C], f32r, kind="ExternalInput").ap()
    wo_d = nc.dram_tensor("wo", [128, HEADS * C], f32r, kind="ExternalInput").ap()
    bb_d = nc.dram_tensor("bb", [128, 2], f32, kind="ExternalInput").ap()
    out_d = nc.dram_tensor("out", [C, N], f32, kind="ExternalOutput").ap()

    with tile.TileContext(nc) as tc, ExitStack() as stack:
        const = stack.enter_context(tc.tile_pool(name="const", bufs=1))

        # DMA order matters: the first k/v matmuls need wk/wv and the first
        # x chunk, so load those first; wq/wo/bb are phase-2-only. x comes in
        # 512-column chunks (both C-blocks per chunk) so compute starts after
        # ~1 chunk instead of after the whole 4 MiB tensor.
        x_sb = const.tile([128, 2 * N], f32r)
        wk_sb = const.tile([128, 2 * HID], f32r)
        nc.sync.dma_start(wk_sb[:], wk_d[:])
        wv_sb = const.tile([128, 2 * HID], f32r)
        nc.sync.dma_start(wv_sb[:], wv_d[:])
        wq_sb = const.tile([128, HEADS * C], f32r)
        wo_sb = const.tile([128, HEADS * C], f32r)
        bb_sb = const.tile([128, 2], f32)
        for cc in range(8):
            for k in range(2):
                nc.sync.dma_start(
                    x_sb[:, k * N + cc * 512 : k * N + (cc + 1) * 512],
                    x_d[k * 128 : (k + 1) * 128, cc * 512 : (cc + 1) * 512],
                )
            if cc == 0:
                nc.sync.dma_start(wq_sb[:], wq_d[:])
            elif cc == 1:
                nc.sync.dma_start(wo_sb[:], wo_d[:])
            elif cc == 2:
                nc.sync.dma_start(bb_sb[:], bb_d[:])

        def xs(k, i):  # lhsT: x rows k-block, spatial tile i -> [128, 128]
            return x_sb[:, k * N + i * 128 : k * N + (i + 1) * 128]

        def xchunk(k, c):  # rhs: x rows k-block, 512-col chunk c
            return x_sb[:, k * N + c * 512 : k * N + (c + 1) * 512]

        rsum = const.tile([128, HEADS], f32)
        ctx_sb = const.tile([128, HEADS * 128], f32r)

        # ---- Phase 1: k^T/v^T projections + exp + context accumulation ----
        # ctx accumulators: one PSUM bank per head (start=True zeroes a whole
        # bank, so heads cannot share one).
        with tc.tile_pool(name="ctxp", bufs=1, space="PSUM") as ctxp, \
             tc.tile_pool(name="pkp", bufs=2, space="PSUM") as pkp, \
             tc.tile_pool(name="pvp", bufs=2, space="PSUM") as pvp, \
             tc.tile_pool(name="ekp", bufs=3) as ekp, \
             tc.tile_pool(name="vtp", bufs=3) as vtp:
            ctx_ps = [
                ctxp.tile([128, 129], f32, name=f"ctx{h}") for h in range(HEADS)
            ]

            def emit_ctx(ek, vt, i):
                for h in range(HEADS):
                    nc.tensor.matmul(
                        ctx_ps[h][:],
                        ek[:, h * 128 : (h + 1) * 128],
                        vt[:, h * 130 : h * 130 + 129],
                        start=(i == 0),
                        stop=(i == NT - 1),
                    )

            pending = None
            for i in range(NT):
                pk = pkp.tile([128, HID], f32, name="pk")
                pv = pvp.tile([128, HID], f32, name="pv")
                for k in range(2):
                    first, last = (k == 0), (k == 1)
                    nc.tensor.matmul(
                        pk[:],
                        xs(k, i),
                        wk_sb[:, k * HID : (k + 1) * HID],
                        start=first,
                        stop=last,
                    )
                    nc.tensor.matmul(
                        pv[:],
                        xs(k, i),
                        wv_sb[:, k * HID : (k + 1) * HID],
                        start=first,
                        stop=last,
                    )
                ek = ekp.tile([128, HID], bf16, name="ek")
                nc.scalar.activation(ek[:], pk[:], AFT.Exp)
                vt = vtp.tile([128, 4 * 130], bf16, name="vt")
                nc.vector.tensor_copy(
                    vt.rearrange("p (h c) -> p h c", c=130)[:, :, 0:128],
                    pv.rearrange("p (h c) -> p h c", c=128),
                )
                nc.gpsimd.memset(
                    vt.rearrange("p (h c) -> p h c", c=130)[:, :, 128:129], 1.0
                )
                # software-pipeline the context matmuls one tile behind so the
                # tensor engine never stalls on the exp/copy of the same tile
                if pending is not None:
                    emit_ctx(*pending)
                pending = (ek, vt, i)
            emit_ctx(*pending)

            # Keep the PE clock warm (HAM K=8/8) across the serial
            # normalize -> M -> W join: throwaway matmuls with no consumers
            # run back-to-back while the other engines work through the join.
            warm = pkp.tile([128, HID], f32, name="pk")
            for _ in range(16):
                nc.tensor.matmul(warm[:], xs(0, 0), wk_sb[:, 0:HID])

            # ---- normalize ctx while the accumulator banks are still open ----
            for h in range(HEADS):
                nc.vector.reciprocal(rsum[:, h : h + 1], ctx_ps[h][:, 128:129])
                nc.vector.tensor_scalar_mul(
                    ctx_sb[:, h * 128 : (h + 1) * 128],
                    ctx_ps[h][:, 0:128],
                    rsum[:, h : h + 1],
                )

        # ---- Phase 2: collapse weights, final matmul ----

        with tc.tile_pool(name="p2p", bufs=2, space="PSUM") as p2p, \
             tc.tile_pool(name="fop", bufs=3) as fop:
            # M_h = ctx_h^T @ w_q_h  -> [128, 256]
            m_sb = const.tile([128, HEADS * C], f32r)
            for h in range(HEADS):
                mp = p2p.tile([128, C], f32, name="mp")
                nc.tensor.matmul(
                    mp[:],
                    ctx_sb[:, h * 128 : (h + 1) * 128],
                    wq_sb[:, h * C : (h + 1) * C],
                )
                nc.vector.tensor_copy(m_sb[:, h * C : (h + 1) * C], mp[:])
            # W^T[ci-block m] = sum_h M_h[:, m-block]^T-contract w_out^T_h
            w_sb = const.tile([128, 2 * C], f32r)
            for m in range(2):
                wp = p2p.tile([128, C], f32, name="wp")
                for h in range(HEADS):
                    nc.tensor.matmul(
                        wp[:],
                        m_sb[:, h * C + m * 128 : h * C + m * 128 + 128],
                        wo_sb[:, h * C : (h + 1) * C],
                        start=(h == 0),
                        stop=(h == HEADS - 1),
                    )
                nc.vector.tensor_copy(w_sb[:, m * C : (m + 1) * C], wp[:])
            # out = W @ x + b, streamed over 8 chunks of 512 columns
            for c in range(8):
                for mo in range(2):
                    fp_ = p2p.tile([128, 512], f32, name="fp")
                    for k in range(2):
                        nc.tensor.matmul(
                            fp_[:],
                            w_sb[:, k * C + mo * 128 : k * C + mo * 128 + 128],
                            xchunk(k, c),
                            start=(k == 0),
                            stop=(k == 1),
                        )
                    fo = fop.tile([128, 512], f32, name=f"fo{mo}")
                    if mo == 0:
                        nc.scalar.activation(
                            fo[:], fp_[:], AFT.Identity, bias=bb_sb[:, 0:1]
                        )
                    else:
                        nc.vector.tensor_scalar_add(fo[:], fp_[:], bb_sb[:, 1:2])
                    nc.sync.dma_start(
                        out_d[mo * 128 : (mo + 1) * 128, c * 512 : (c + 1) * 512],
                        fo[:],
                    )

    nc.compile()
    return nc


def _get_program():
    if "nc" not in _BUILD_CACHE:
        _BUILD_CACHE["nc"] = _build_program()
    return _BUILD_CACHE["nc"]


def _pack_weights(w_qkv, w_out, b_out):
    w_q = np.ascontiguousarray(w_qkv[0:HID]).astype(np.float32)  # [512, 256]
    w_k = w_qkv[HID : 2 * HID]
    w_v = w_qkv[2 * HID : 3 * HID]

    def pack_T(w):  # w [512, 256] -> w.T [256, 512] -> [128, 2*512]
        return np.ascontiguousarray(
            w.T.reshape(2, 128, HID).transpose(1, 0, 2).reshape(128, 2 * HID)
        ).astype(np.float32)

    def pack_rows(w):  # w [512, 256] -> [128, 4*256], block h = rows h*128:+128
        return np.ascontiguousarray(
            w.reshape(HEADS, 128, C).transpose(1, 0, 2).reshape(128, HEADS * C)
        ).astype(np.float32)

    return {
        "wk": pack_T(w_k),
        "wv": pack_T(w_v),
        "wq": pack_rows(w_q),
        "wo": pack_rows(np.ascontiguousarray(w_out.T)),  # w_out.T [512, 256]
        "bb": np.ascontiguousarray(b_out.reshape(2, 128).T).astype(np.float32),
    }


def kernel(x, w_qkv, w_out, b_out):
    from concourse.bass_utils import run_bass_kernel_spmd

    x = np.asarray(x, dtype=np.float32)
    B = x.shape[0]
    assert B == NCORES and x.shape[1:] == (C, 64, 64)

    nc = _get_program()
    packed = _pack_weights(
        np.asarray(w_qkv, np.float32),
        np.asarray(w_out, np.float32),
        np.asarray(b_out, np.float32),
    )
    in_maps = [
        {"x": np.ascontiguousarray(x[b].reshape(C, N)), **packed} for b in range(B)
    ]
    res = run_bass_kernel_spmd(nc, in_maps, core_ids=list(range(NCORES)))
    out = np.stack([res.results[b]["out"] for b in range(B)], axis=0)
    return out.reshape(B, C, 64, 64).astype(np.float32)
